# revision 21
# baseline (speedup 1.0000x reference)
"""Izhikevich spiking-neuron scan on 8 Trainium2 NeuronCores.

Problem: x[512, 65536] f32 input currents; per reference step t:
    v' = (4v^2 + 5v + 1.4 - r + x_t) * DT          (DT = 1/512)
    r' = A*(B-1)*DT * v'  (= K*v', memoryless given new v')
    fire = v' >= 0.3;  v' <- C, r' <- r' + D where fire
    out[t] = fire
Sharding: neurons (axis 1) split 8 ways; each core handles 8192 neurons.

Fast path (no neuron can ever fire -- true whenever max|x| < ~20, since
v stays ~ (1.4+x)/512 << 0.3): absent fires, r_t = K*v_t exactly, so

    v_{t+1} = a*v_t + b_t + q_t,   a = (5-K)*DT,  b_t = DT*(1.4 + x_t),
    q_t = 4*DT*v_t^2.

The quadratic term is ~0.03%% of v and the map is contractive (|a|~0.01),
so one Neumann-series estimate  v0_t = b_t + a*b_{t-1}  makes
q_t ~= 4*DT*v0_t^2 accurate to ~1e-8 rel of v (below fp32 eps), after
which the recurrence is EXACTLY affine and solved by the DVE hardware
scan instruction (tensor_tensor_scan: state = a*state + b2, fp32).

Layout: host transposes x to neuron-major and pads each neuron's 512
columns with a 6-col halo [z z z z z I]: z-cols have x=-1.4 (b=0, decays
any carried state to <1e-12), I-col has x = C'/DT-1.4 with C' = 5C/(5-K)
so the scan state lands exactly on the reference's first step
(v_1 = (4C^2+5C+1.4-0+x_0)*DT, r_0=0). Each partition holds 64 neurons'
column streams; chunks of 4 neurons (2072 cols) are processed per
instruction with a 2-col input overlap so shifted operands never cross
tile boundaries. Output m = (v >= 0.3) as u8, un-padded on host.

Per chunk (engine assignment tuned for balance):
  b  = DT*x + 1.4*DT            Act (Copy w/ scale+bias)
  v0 = a*b_{-1} + b             Pool scalar_tensor_tensor
  q  = Square(sqrt(4DT)*v0)     Act
  b2 = q_{-1} + b               DVE/Pool tensor_tensor (split by chunk)
  v  = scan(a, b2)              DVE tensor_tensor_scan
  m  = (v >= THRESH) -> u8      DVE tensor_scalar

Fallback path (any |x| >= 20): original stepwise kernel, unconditionally
exact including fire/reset handling.
"""

import math
import os
import sys

import numpy as np

if "/opt/trn_rl_repo" not in sys.path:
    sys.path.insert(0, "/opt/trn_rl_repo")

# ---- problem constants (hardcoded; kernel.py must be self-contained) ----
T = 512
N = 65536
NCORES = 8
NLOC = N // NCORES          # 8192 neurons per core
P = 128                     # SBUF partitions
JPER = NLOC // P            # 64 neurons per partition

A = 0.02
B = 0.2
C = -0.065
D = 0.008
DT = 1.0 / T
THRESH = 0.3

K = A * (B - 1.0) * DT                      # r_t = K * v_t  (t >= 1)
A_LIN = np.float32((5.0 - K) * DT)          # affine decay per step
S_X = np.float32(DT)                        # x forcing scale
C_F = 1.4 * DT                              # constant forcing
PHI = C_F / (1.0 - float(A_LIN))            # linear-part fixed point
CS = np.float32(1.4)                        # c/s: data1 = x + 1.4 (h units)
# multiplier scan: v_t = D_t*v_{t-1} + s*x_t + c, D_t = a + 4*DT*vhat_{t-1},
# vhat = s*x + PHI (linear one-step estimate; quadratic term is 0.03% of v)
D_SCALE = np.float32(4.0 * DT * DT)         # D = D_SCALE*x_{t-1} + D_BIAS
D_BIAS = np.float32(A_LIN + np.float32(4.0 * DT) * np.float32(PHI))
H_TH = np.float32(THRESH / DT)              # threshold in h = v/s units
# init-col plants the reference initial state exactly:
# scan linear term gives D*C'; reference's t=0 step (r_0=0) needs 5*DT*C
# =>  C' = 5C/(5-K). During zero cols state sits at PHIP (fixed point of
# state = D_BIAS*state + c); the init col then lands exactly on C'.
C_INIT = 5.0 * C / (5.0 - K)
PHIP = C_F / (1.0 - float(D_BIAS))
X_ICOL = np.float32((C_INIT - float(D_BIAS) * PHIP - C_F) / float(S_X))

W = 6                                        # halo cols per neuron
COLS = W + T                                 # 518
FTOT = JPER * COLS                           # 33152 cols per partition
JC = int(os.environ.get("IZI_JC", "2"))      # neurons per chunk
FC = JC * COLS                               # 2072
NCH = JPER // JC                             # 16 chunks
NBUFS = int(os.environ.get("IZI_NBUFS", "6"))
XGRP = int(os.environ.get("IZI_XGRP", "1"))   # chunks per input DMA
XBUFS = int(os.environ.get("IZI_XBUFS", "6"))

X_SAFE_BOUND = 20.0                          # |x| < this ==> no fire possible

# engine split knobs (sim-tuned). Each is a cycle of engine names indexed
# by chunk: e.g. "pool,pool,dve" puts chunk 0,1 on pool, 2 on dve, ...
def _cycle(env, default):
    return os.environ.get(env, default).split(",")


XC_ENGS = _cycle("IZI_XC", "pool")
D_ENGS = _cycle("IZI_D", "act")
SCAN_ENGS = _cycle("IZI_SCAN", "dve")
M_ENGS = _cycle("IZI_M", "dve")


def _build_nc_scan():
    import concourse.bacc as bacc
    import concourse.mybir as mybir
    from concourse import tile

    fp32 = mybir.dt.float32
    u8 = mybir.dt.uint8
    op = mybir.AluOpType
    act = mybir.ActivationFunctionType

    nc = bacc.Bacc("TRN2", target_bir_lowering=False)
    x_d = nc.dram_tensor("x", [P, FTOT + 2], fp32, kind="ExternalInput")
    m_d = nc.dram_tensor("spk", [P, FTOT], u8, kind="ExternalOutput")

    def eng(cyc, ci):
        name = cyc[ci % len(cyc)]
        return {"dve": nc.vector, "pool": nc.gpsimd, "act": nc.scalar}[name], name

    with tile.TileContext(nc) as tc:
        with (
            tc.tile_pool(name="xin", bufs=XBUFS) as xin_pool,
            tc.tile_pool(name="xc", bufs=NBUFS) as xc_pool,
            tc.tile_pool(name="dd", bufs=NBUFS) as d_pool,
            tc.tile_pool(name="hh", bufs=NBUFS) as h_pool,
            tc.tile_pool(name="mm", bufs=NBUFS) as m_pool,
            tc.tile_pool(name="const", bufs=1) as c_pool,
        ):
            mthb = c_pool.tile([P, 1], fp32, tag="mthb")
            nc.vector.memset(mthb[:], -float(H_TH))

            xg = None
            for ci in range(NCH):
                if ci % XGRP == 0:
                    ng = min(XGRP, NCH - ci)
                    xg = xin_pool.tile([P, ng * FC + 2], fp32, tag="x")
                    nc.sync.dma_start(
                        out=xg[:], in_=x_d[:, ci * FC : (ci + ng) * FC + 2]
                    )
                off = (ci % XGRP) * FC
                xt = xg[:, off : off + FC + 2]
                # xc[j] = x[j+2] + 1.4      (FC cols, chunk-aligned, h units)
                xct = xc_pool.tile([P, FC], fp32, tag="xc")
                e, _ = eng(XC_ENGS, ci)
                e.tensor_scalar(xct[:], xt[:, 2 : FC + 2], float(CS), None, op.add)
                # D[j] = D_SCALE*x[j+1] + D_BIAS   (multiplier, reads x_{t-1})
                dt_ = d_pool.tile([P, FC], fp32, tag="D")
                e, nm = eng(D_ENGS, ci)
                if nm == "act":
                    nc.scalar.activation(
                        dt_[:], xt[:, 1 : FC + 1], act.Copy,
                        bias=float(D_BIAS), scale=float(D_SCALE),
                    )
                else:
                    e.tensor_scalar(
                        dt_[:], xt[:, 1 : FC + 1], float(D_SCALE), float(D_BIAS),
                        op.mult, op.add,
                    )
                # h = scan: state = D*state + xc   (h = v/DT units)
                ht = h_pool.tile([P, FC], fp32, tag="h")
                e, _ = eng(SCAN_ENGS, ci)
                e.tensor_tensor_scan(ht[:], dt_[:], xct[:], 0.0, op.mult, op.add)
                # m = (h >= THRESH/DT) as u8
                mt = m_pool.tile([P, FC], u8, tag="m")
                e, nm = eng(M_ENGS, ci)
                if nm == "act":
                    # Sign(h - H_TH) in {-1,0,1}; u8 saturating cast -> {0,1}
                    nc.scalar.activation(
                        mt[:], ht[:], act.Sign, bias=mthb[:], scale=1.0
                    )
                else:
                    e.tensor_scalar(mt[:], ht[:], float(H_TH), None, op.is_ge)
                nc.sync.dma_start(
                    out=m_d[:, ci * FC : (ci + 1) * FC], in_=mt[:]
                )
    nc.compile()
    return nc


# ---------------- fallback: original stepwise kernel (exact w/ fires) ----
F_SW = NLOC // P
TC_SW = 64
NCHUNK_SW = T // TC_SW
_beta0 = 320.0 - 25.0 / 16.0 + 1.4
_Thg = THRESH / DT + 320.0
_Rg = C / DT + 320.0
_Rsg = math.sqrt(_Rg * _Rg - D / (4.0 * DT * DT))
_sigma = 1.0 / (_Thg - _Rsg)
C4 = np.float32(4.0 * DT * DT / _sigma)
C_R = np.float32(-K * DT)
TH_S = np.float32(_sigma * _Thg)
G0 = np.float32(_sigma * _Rg)
PRE_SCALE = np.float32(_sigma)
PRE_BIAS = np.float32(_sigma * (_beta0 + 320.0 * K * DT))


def _build_nc_stepwise():
    import concourse.bacc as bacc
    import concourse.mybir as mybir
    from concourse import tile

    fp32 = mybir.dt.float32
    op = mybir.AluOpType

    nc = bacc.Bacc("TRN2", target_bir_lowering=False)
    x_d = nc.dram_tensor("x", [T, NLOC], fp32, kind="ExternalInput")
    y_d = nc.dram_tensor("spk", [T, NLOC], fp32, kind="ExternalOutput")

    def chunk_view(dram, ci):
        return dram[ci * TC_SW : (ci + 1) * TC_SW, :].rearrange(
            "t (p f) -> p t f", p=P
        )

    with tile.TileContext(nc) as tc:
        with (
            tc.tile_pool(name="xin", bufs=2) as xin_pool,
            tc.tile_pool(name="pre", bufs=2) as pre_pool,
            tc.tile_pool(name="out", bufs=2) as out_pool,
            tc.tile_pool(name="state", bufs=2) as g_pool,
            tc.tile_pool(name="gp", bufs=2) as gp_pool,
            tc.tile_pool(name="q", bufs=2) as q_pool,
            tc.tile_pool(name="w", bufs=2) as w_pool,
        ):
            pre_tiles = [None] * NCHUNK_SW

            def load_chunk(ci):
                xt = xin_pool.tile([P, TC_SW * F_SW], fp32, tag="xin")
                nc.sync.dma_start(
                    out=xt.rearrange("p (t f) -> p t f", t=TC_SW),
                    in_=chunk_view(x_d, ci),
                )
                pt = pre_pool.tile([P, TC_SW * F_SW], fp32, tag="pre")
                nc.scalar.activation(
                    pt[:], xt[:],
                    mybir.ActivationFunctionType.Copy,
                    bias=float(PRE_BIAS), scale=float(PRE_SCALE),
                )
                pre_tiles[ci] = pt

            G = g_pool.tile([P, F_SW], fp32, tag="G")
            nc.vector.memset(G[:], float(G0))
            load_chunk(0)
            w = None

            for ci in range(NCHUNK_SW):
                if ci + 1 < NCHUNK_SW:
                    load_chunk(ci + 1)
                pre = pre_tiles[ci]
                ot = out_pool.tile([P, TC_SW * F_SW], fp32, tag="out")
                for tt in range(TC_SW):
                    t = ci * TC_SW + tt
                    win = pre[:, 0:F_SW] if t == 0 else w[:]
                    q = q_pool.tile([P, F_SW], fp32, tag="q")
                    nc.vector.tensor_tensor(q[:], G[:], G[:], op.mult)
                    Gp = gp_pool.tile([P, F_SW], fp32, tag="Gp")
                    nc.vector.scalar_tensor_tensor(
                        Gp[:], q[:], float(C4), win, op.mult, op.add
                    )
                    m = ot[:, tt * F_SW : (tt + 1) * F_SW]
                    nc.vector.tensor_scalar(
                        m, Gp[:], float(TH_S), None, op.is_ge
                    )
                    if t + 1 < T:
                        if tt + 1 < TC_SW:
                            nxt = pre[:, (tt + 1) * F_SW : (tt + 2) * F_SW]
                        else:
                            nxt = pre_tiles[ci + 1][:, 0:F_SW]
                        w = w_pool.tile([P, F_SW], fp32, tag="w")
                        nc.vector.scalar_tensor_tensor(
                            w[:], Gp[:], float(C_R), nxt, op.mult, op.add
                        )
                        G = g_pool.tile([P, F_SW], fp32, tag="G")
                        nc.vector.scalar_tensor_tensor(
                            G[:], Gp[:], float(TH_S), m, op.min, op.subtract
                        )
                pre_tiles[ci] = None
                nc.sync.dma_start(
                    out=chunk_view(y_d, ci),
                    in_=ot.rearrange("p (t f) -> p t f", t=TC_SW),
                )
    nc.compile()
    return nc


_CACHE: dict = {}


def _pack_core(xc: np.ndarray) -> np.ndarray:
    """x slice [T, NLOC] f32 -> device layout [P, FTOT+2] with halos."""
    xh = np.empty((P, FTOT + 2), dtype=np.float32)
    xh[:, :2] = 0.0
    body = xh[:, 2:].reshape(P, JPER, COLS)
    body[:, :, : W - 1] = 0.0
    body[:, :, W - 1] = X_ICOL
    body[:, :, W:] = np.ascontiguousarray(xc.T).reshape(P, JPER, T)
    return xh


def _unpack_core(mh: np.ndarray) -> np.ndarray:
    """device output [P, FTOT] u8 -> [T, NLOC] u8."""
    return mh.reshape(P, JPER, COLS)[:, :, W:].reshape(NLOC, T).T


def kernel(x: np.ndarray) -> np.ndarray:
    from concourse.bass_utils import run_bass_kernel_spmd

    x = np.ascontiguousarray(np.asarray(x, np.float32))
    assert x.shape == (T, N), x.shape

    core_ids = list(range(NCORES))
    if float(np.max(np.abs(x))) < X_SAFE_BOUND:
        if "scan" not in _CACHE:
            _CACHE["scan"] = _build_nc_scan()
        nc = _CACHE["scan"]
        in_maps = [
            {"x": _pack_core(x[:, c * NLOC : (c + 1) * NLOC])}
            for c in core_ids
        ]
        res = run_bass_kernel_spmd(nc, in_maps, core_ids)
        outs = [_unpack_core(res.results[c]["spk"]) for c in core_ids]
        return np.concatenate(outs, axis=1).astype(np.float32)

    # fallback: stepwise kernel, exact under firing
    if "sw" not in _CACHE:
        _CACHE["sw"] = _build_nc_stepwise()
    nc = _CACHE["sw"]
    in_maps = [
        {"x": np.ascontiguousarray(x[:, c * NLOC : (c + 1) * NLOC])}
        for c in core_ids
    ]
    res = run_bass_kernel_spmd(nc, in_maps, core_ids)
    return np.concatenate(
        [res.results[c]["spk"] for c in core_ids], axis=1
    )


if __name__ == "__main__":
    xt = np.random.randn(T, N).astype(np.float32)
    y = kernel(xt)
    print("out", y.shape, y.dtype, y.sum())


# revision 22
# speedup vs baseline: 5.0175x; 5.0175x over previous
"""Izhikevich spiking-neuron scan on 8 Trainium2 NeuronCores.

Problem: x[512, 65536] f32 input currents; per reference step t:
    v' = (4v^2 + 5v + 1.4 - r + x_t) * DT          (DT = 1/512)
    r' = A*(B-1)*DT * v'  (= K*v', memoryless given new v')
    fire = v' >= 0.3;  v' <- C, r' <- r' + D where fire
    out[t] = fire
Sharding: neurons (axis 1) split 8 ways; each core handles 8192 neurons.

Fast path (no neuron can ever fire -- true whenever max|x| < ~20, since
v stays ~ (1.4+x)/512 << 0.3): absent fires, r_t = K*v_t exactly, so

    v_{t+1} = a*v_t + b_t + q_t,   a = (5-K)*DT,  b_t = DT*(1.4 + x_t),
    q_t = 4*DT*v_t^2.

The quadratic term is ~0.03%% of v and the map is contractive (|a|~0.01),
so one Neumann-series estimate  v0_t = b_t + a*b_{t-1}  makes
q_t ~= 4*DT*v0_t^2 accurate to ~1e-8 rel of v (below fp32 eps), after
which the recurrence is EXACTLY affine and solved by the DVE hardware
scan instruction (tensor_tensor_scan: state = a*state + b2, fp32).

Layout: host transposes x to neuron-major and pads each neuron's 512
columns with a 6-col halo [z z z z z I]: z-cols have x=-1.4 (b=0, decays
any carried state to <1e-12), I-col has x = C'/DT-1.4 with C' = 5C/(5-K)
so the scan state lands exactly on the reference's first step
(v_1 = (4C^2+5C+1.4-0+x_0)*DT, r_0=0). Each partition holds 64 neurons'
column streams; chunks of 4 neurons (2072 cols) are processed per
instruction with a 2-col input overlap so shifted operands never cross
tile boundaries. Output m = (v >= 0.3) as u8, un-padded on host.

Per chunk (engine assignment tuned for balance):
  b  = DT*x + 1.4*DT            Act (Copy w/ scale+bias)
  v0 = a*b_{-1} + b             Pool scalar_tensor_tensor
  q  = Square(sqrt(4DT)*v0)     Act
  b2 = q_{-1} + b               DVE/Pool tensor_tensor (split by chunk)
  v  = scan(a, b2)              DVE tensor_tensor_scan
  m  = (v >= THRESH) -> u8      DVE tensor_scalar

Fallback path (any |x| >= 20): original stepwise kernel, unconditionally
exact including fire/reset handling.
"""

import math
import os
import sys

import numpy as np

if "/opt/trn_rl_repo" not in sys.path:
    sys.path.insert(0, "/opt/trn_rl_repo")

# ---- problem constants (hardcoded; kernel.py must be self-contained) ----
T = 512
N = 65536
NCORES = 8
NLOC = N // NCORES          # 8192 neurons per core
P = 128                     # SBUF partitions
JPER = NLOC // P            # 64 neurons per partition

A = 0.02
B = 0.2
C = -0.065
D = 0.008
DT = 1.0 / T
THRESH = 0.3

K = A * (B - 1.0) * DT                      # r_t = K * v_t  (t >= 1)
A_LIN = np.float32((5.0 - K) * DT)          # affine decay per step
S_X = np.float32(DT)                        # x forcing scale
C_F = 1.4 * DT                              # constant forcing
PHI = C_F / (1.0 - float(A_LIN))            # linear-part fixed point
CS = np.float32(1.4)                        # c/s: data1 = x + 1.4 (h units)
# multiplier scan: v_t = D_t*v_{t-1} + s*x_t + c, D_t = a + 4*DT*vhat_{t-1},
# vhat = s*x + PHI (linear one-step estimate; quadratic term is 0.03% of v)
D_SCALE = np.float32(4.0 * DT * DT)         # D = D_SCALE*x_{t-1} + D_BIAS
D_BIAS = np.float32(A_LIN + np.float32(4.0 * DT) * np.float32(PHI))
H_TH = np.float32(THRESH / DT)              # threshold in h = v/s units
# init-col plants the reference initial state exactly:
# scan linear term gives D*C'; reference's t=0 step (r_0=0) needs 5*DT*C
# =>  C' = 5C/(5-K). During zero cols state sits at PHIP (fixed point of
# state = D_BIAS*state + c); the init col then lands exactly on C'.
C_INIT = 5.0 * C / (5.0 - K)
PHIP = C_F / (1.0 - float(D_BIAS))
X_ICOL = np.float32((C_INIT - float(D_BIAS) * PHIP - C_F) / float(S_X))

W = 6                                        # halo cols per neuron
COLS = W + T                                 # 518
FTOT = JPER * COLS                           # 33152 cols per partition
JC = int(os.environ.get("IZI_JC", "2"))      # neurons per chunk
FC = JC * COLS                               # 2072
NCH = JPER // JC                             # 16 chunks
NBUFS = int(os.environ.get("IZI_NBUFS", "6"))
XGRP = int(os.environ.get("IZI_XGRP", "1"))   # chunks per input DMA
XBUFS = int(os.environ.get("IZI_XBUFS", "6"))

X_SAFE_BOUND = 20.0                          # |x| < this ==> no fire possible

# engine split knobs (sim-tuned). Each is a cycle of engine names indexed
# by chunk: e.g. "pool,pool,dve" puts chunk 0,1 on pool, 2 on dve, ...
def _cycle(env, default):
    return os.environ.get(env, default).split(",")


XC_ENGS = _cycle("IZI_XC", "pool")
D_ENGS = _cycle("IZI_D", "act")
SCAN_ENGS = _cycle("IZI_SCAN", "dve")
M_ENGS = _cycle("IZI_M", "dve")


def _build_nc_scan():
    import concourse.bacc as bacc
    import concourse.mybir as mybir
    from concourse import tile

    fp32 = mybir.dt.float32
    u8 = mybir.dt.uint8
    op = mybir.AluOpType
    act = mybir.ActivationFunctionType

    nc = bacc.Bacc("TRN2", target_bir_lowering=False)
    x_d = nc.dram_tensor("x", [P, FTOT + 2], fp32, kind="ExternalInput")
    m_d = nc.dram_tensor("spk", [P, FTOT], u8, kind="ExternalOutput")

    def eng(cyc, ci):
        name = cyc[ci % len(cyc)]
        return {"dve": nc.vector, "pool": nc.gpsimd, "act": nc.scalar}[name], name

    with tile.TileContext(nc) as tc:
        with (
            tc.tile_pool(name="xin", bufs=XBUFS) as xin_pool,
            tc.tile_pool(name="xc", bufs=NBUFS) as xc_pool,
            tc.tile_pool(name="dd", bufs=NBUFS) as d_pool,
            tc.tile_pool(name="hh", bufs=NBUFS) as h_pool,
            tc.tile_pool(name="mm", bufs=NBUFS) as m_pool,
            tc.tile_pool(name="const", bufs=1) as c_pool,
        ):
            mthb = c_pool.tile([P, 1], fp32, tag="mthb")
            nc.vector.memset(mthb[:], -float(H_TH))

            xg = None
            for ci in range(NCH):
                if ci % XGRP == 0:
                    ng = min(XGRP, NCH - ci)
                    xg = xin_pool.tile([P, ng * FC + 2], fp32, tag="x")
                    nc.sync.dma_start(
                        out=xg[:], in_=x_d[:, ci * FC : (ci + ng) * FC + 2]
                    )
                off = (ci % XGRP) * FC
                xt = xg[:, off : off + FC + 2]
                # xc[j] = x[j+2] + 1.4      (FC cols, chunk-aligned, h units)
                xct = xc_pool.tile([P, FC], fp32, tag="xc")
                e, nm = eng(XC_ENGS, ci)
                if nm == "act":
                    nc.scalar.activation(
                        xct[:], xt[:, 2 : FC + 2], act.Copy,
                        bias=float(CS), scale=1.0,
                    )
                else:
                    e.tensor_scalar(
                        xct[:], xt[:, 2 : FC + 2], float(CS), None, op.add
                    )
                # D[j] = D_SCALE*x[j+1] + D_BIAS   (multiplier, reads x_{t-1})
                dt_ = d_pool.tile([P, FC], fp32, tag="D")
                e, nm = eng(D_ENGS, ci)
                if nm == "act":
                    nc.scalar.activation(
                        dt_[:], xt[:, 1 : FC + 1], act.Copy,
                        bias=float(D_BIAS), scale=float(D_SCALE),
                    )
                else:
                    e.tensor_scalar(
                        dt_[:], xt[:, 1 : FC + 1], float(D_SCALE), float(D_BIAS),
                        op.mult, op.add,
                    )
                # h = scan: state = D*state + xc   (h = v/DT units)
                ht = h_pool.tile([P, FC], fp32, tag="h")
                e, _ = eng(SCAN_ENGS, ci)
                e.tensor_tensor_scan(ht[:], dt_[:], xct[:], 0.0, op.mult, op.add)
                # m = (h >= THRESH/DT) as u8
                mt = m_pool.tile([P, FC], u8, tag="m")
                e, nm = eng(M_ENGS, ci)
                if nm == "act":
                    # Sign(h - H_TH) in {-1,0,1}; u8 saturating cast -> {0,1}
                    nc.scalar.activation(
                        mt[:], ht[:], act.Sign, bias=mthb[:], scale=1.0
                    )
                else:
                    e.tensor_scalar(mt[:], ht[:], float(H_TH), None, op.is_ge)
                nc.sync.dma_start(
                    out=m_d[:, ci * FC : (ci + 1) * FC], in_=mt[:]
                )
    nc.compile()
    return nc


# ---------------- fallback: original stepwise kernel (exact w/ fires) ----
F_SW = NLOC // P
TC_SW = 64
NCHUNK_SW = T // TC_SW
_beta0 = 320.0 - 25.0 / 16.0 + 1.4
_Thg = THRESH / DT + 320.0
_Rg = C / DT + 320.0
_Rsg = math.sqrt(_Rg * _Rg - D / (4.0 * DT * DT))
_sigma = 1.0 / (_Thg - _Rsg)
C4 = np.float32(4.0 * DT * DT / _sigma)
C_R = np.float32(-K * DT)
TH_S = np.float32(_sigma * _Thg)
G0 = np.float32(_sigma * _Rg)
PRE_SCALE = np.float32(_sigma)
PRE_BIAS = np.float32(_sigma * (_beta0 + 320.0 * K * DT))


def _build_nc_stepwise():
    import concourse.bacc as bacc
    import concourse.mybir as mybir
    from concourse import tile

    fp32 = mybir.dt.float32
    op = mybir.AluOpType

    nc = bacc.Bacc("TRN2", target_bir_lowering=False)
    x_d = nc.dram_tensor("x", [T, NLOC], fp32, kind="ExternalInput")
    y_d = nc.dram_tensor("spk", [T, NLOC], fp32, kind="ExternalOutput")

    def chunk_view(dram, ci):
        return dram[ci * TC_SW : (ci + 1) * TC_SW, :].rearrange(
            "t (p f) -> p t f", p=P
        )

    with tile.TileContext(nc) as tc:
        with (
            tc.tile_pool(name="xin", bufs=2) as xin_pool,
            tc.tile_pool(name="pre", bufs=2) as pre_pool,
            tc.tile_pool(name="out", bufs=2) as out_pool,
            tc.tile_pool(name="state", bufs=2) as g_pool,
            tc.tile_pool(name="gp", bufs=2) as gp_pool,
            tc.tile_pool(name="q", bufs=2) as q_pool,
            tc.tile_pool(name="w", bufs=2) as w_pool,
        ):
            pre_tiles = [None] * NCHUNK_SW

            def load_chunk(ci):
                xt = xin_pool.tile([P, TC_SW * F_SW], fp32, tag="xin")
                nc.sync.dma_start(
                    out=xt.rearrange("p (t f) -> p t f", t=TC_SW),
                    in_=chunk_view(x_d, ci),
                )
                pt = pre_pool.tile([P, TC_SW * F_SW], fp32, tag="pre")
                nc.scalar.activation(
                    pt[:], xt[:],
                    mybir.ActivationFunctionType.Copy,
                    bias=float(PRE_BIAS), scale=float(PRE_SCALE),
                )
                pre_tiles[ci] = pt

            G = g_pool.tile([P, F_SW], fp32, tag="G")
            nc.vector.memset(G[:], float(G0))
            load_chunk(0)
            w = None

            for ci in range(NCHUNK_SW):
                if ci + 1 < NCHUNK_SW:
                    load_chunk(ci + 1)
                pre = pre_tiles[ci]
                ot = out_pool.tile([P, TC_SW * F_SW], fp32, tag="out")
                for tt in range(TC_SW):
                    t = ci * TC_SW + tt
                    win = pre[:, 0:F_SW] if t == 0 else w[:]
                    q = q_pool.tile([P, F_SW], fp32, tag="q")
                    nc.vector.tensor_tensor(q[:], G[:], G[:], op.mult)
                    Gp = gp_pool.tile([P, F_SW], fp32, tag="Gp")
                    nc.vector.scalar_tensor_tensor(
                        Gp[:], q[:], float(C4), win, op.mult, op.add
                    )
                    m = ot[:, tt * F_SW : (tt + 1) * F_SW]
                    nc.vector.tensor_scalar(
                        m, Gp[:], float(TH_S), None, op.is_ge
                    )
                    if t + 1 < T:
                        if tt + 1 < TC_SW:
                            nxt = pre[:, (tt + 1) * F_SW : (tt + 2) * F_SW]
                        else:
                            nxt = pre_tiles[ci + 1][:, 0:F_SW]
                        w = w_pool.tile([P, F_SW], fp32, tag="w")
                        nc.vector.scalar_tensor_tensor(
                            w[:], Gp[:], float(C_R), nxt, op.mult, op.add
                        )
                        G = g_pool.tile([P, F_SW], fp32, tag="G")
                        nc.vector.scalar_tensor_tensor(
                            G[:], Gp[:], float(TH_S), m, op.min, op.subtract
                        )
                pre_tiles[ci] = None
                nc.sync.dma_start(
                    out=chunk_view(y_d, ci),
                    in_=ot.rearrange("p (t f) -> p t f", t=TC_SW),
                )
    nc.compile()
    return nc


_CACHE: dict = {}


def _pack_core(xc: np.ndarray) -> np.ndarray:
    """x slice [T, NLOC] f32 -> device layout [P, FTOT+2] with halos."""
    xh = np.empty((P, FTOT + 2), dtype=np.float32)
    xh[:, :2] = 0.0
    body = xh[:, 2:].reshape(P, JPER, COLS)
    body[:, :, : W - 1] = 0.0
    body[:, :, W - 1] = X_ICOL
    body[:, :, W:] = np.ascontiguousarray(xc.T).reshape(P, JPER, T)
    return xh


def _unpack_core(mh: np.ndarray) -> np.ndarray:
    """device output [P, FTOT] u8 -> [T, NLOC] u8."""
    return mh.reshape(P, JPER, COLS)[:, :, W:].reshape(NLOC, T).T


def kernel(x: np.ndarray) -> np.ndarray:
    from concourse.bass_utils import run_bass_kernel_spmd

    x = np.ascontiguousarray(np.asarray(x, np.float32))
    assert x.shape == (T, N), x.shape

    core_ids = list(range(NCORES))
    if float(np.max(np.abs(x))) < X_SAFE_BOUND:
        if "scan" not in _CACHE:
            _CACHE["scan"] = _build_nc_scan()
        nc = _CACHE["scan"]
        in_maps = [
            {"x": _pack_core(x[:, c * NLOC : (c + 1) * NLOC])}
            for c in core_ids
        ]
        res = run_bass_kernel_spmd(nc, in_maps, core_ids)
        outs = [_unpack_core(res.results[c]["spk"]) for c in core_ids]
        return np.concatenate(outs, axis=1).astype(np.float32)

    # fallback: stepwise kernel, exact under firing
    if "sw" not in _CACHE:
        _CACHE["sw"] = _build_nc_stepwise()
    nc = _CACHE["sw"]
    in_maps = [
        {"x": np.ascontiguousarray(x[:, c * NLOC : (c + 1) * NLOC])}
        for c in core_ids
    ]
    res = run_bass_kernel_spmd(nc, in_maps, core_ids)
    return np.concatenate(
        [res.results[c]["spk"] for c in core_ids], axis=1
    )


if __name__ == "__main__":
    xt = np.random.randn(T, N).astype(np.float32)
    y = kernel(xt)
    print("out", y.shape, y.dtype, y.sum())
